# revision 13
# baseline (speedup 1.0000x reference)
"""CosineTransformerBlock Trainium2 kernel (8 NeuronCores, SPMD + pair AllGather).

Sharding: core c handles batch b = c // 2 and token rows
[ (c % 2) * 1024 : (c % 2) * 1024 + 1024 ] of that batch — for Q, K AND V.
Cosine attention has no softmax, so
    (qn @ kn^T) @ v  ==  qn @ (kn^T @ v)
and M_h = kn_h^T v_h is a sum over kv tokens: each core computes the
partial M over its own 1024 kv tokens and the pair {2b, 2b+1} merges via
a tiny AllGather (128 KB) + on-chip add.  The Q-side projection work is
interleaved into phase 1 so the AllGather latency hides behind it.

Precision: attention path in bf16 (fp32 PSUM); FFN in fp8 e4m3 with
DoubleRow (double-pumped) matmuls — w1/w2 are absmax-scaled to e4m3 on
the host, the dequant scale is folded into the gelu evacuation
(gelu(psum/sw1 + bff1)) and into the final fused (psum/sw2 + x)
scalar_tensor_tensor evacuation.  LN outputs and gelu outputs are cast
to e4m3 with scale 1 (values sit in e4m3's normal range).  Q and the
residual x live in bf16 (rel-err impact ~1e-4).

Layout/perf:
  - all weights SBUF-resident, ~16 large DMAs, no streaming;
  - full-tile XBAR transposes (1 instruction per [128,1024] tile);
  - input loads on the gpsimd (SWDGE) queue, transposes on SP, so
    neither blocks the other (in-order queues);
  - one activation-table switch (sqrt family -> gelu family);
  - K/V/Q projection PSUMs share one ring; M accumulated in PSUM across
    all 8 kv tiles with a one-tile software pipeline.
"""

import os
import sys

sys.path.insert(0, "/opt/trn_rl_repo")

import numpy as np
import ml_dtypes

# ---- problem shapes (hardcoded per contract) ----
B, N, D = 4, 2048, 1024
H, DH = 16, 64
INNER = H * DH  # 1024
MLP = 4096
EPS = 1e-5
NCORES = 8
TQ = N // 2  # 1024 query tokens per core
TKV = N // 2  # 1024 kv tokens per core (pair-deduplicated)
P = 128
DC = D // P  # 8
IC = INNER // P  # 8
MC = MLP // P  # 32
NQT = TQ // P  # 8
NKT = TKV // P  # 8
GT = 4  # token tiles per FFN group
NG = NQT // GT  # 2

F8MAX = 224.0  # headroom under e4m3 max-normal 240

BF16 = None
F32 = None
F8 = None


def _dt():
    global BF16, F32, F8
    import concourse.mybir as mybir

    BF16 = mybir.dt.bfloat16
    F32 = mybir.dt.float32
    F8 = mybir.dt.float8e4
    return mybir


def build_nc(bias_rows, sw1, sw2):
    """bias_rows: host fp32 rows (bq,bk,bv,bo,b2s); a K=1 ones-matmul is
    emitted per nonzero row.  sw1/sw2: fp8 weight scales."""
    mybir = _dt()
    import concourse.tile as tile
    from concourse import bacc

    AF = mybir.ActivationFunctionType
    ALU = mybir.AluOpType

    nc = bacc.Bacc("TRN2", target_bir_lowering=False, debug=False, num_devices=NCORES)

    # ---- DRAM I/O ----
    Qd = nc.dram_tensor("q_tok", [TQ, D], BF16, kind="ExternalInput").ap()
    Kd = nc.dram_tensor("k_tok", [TKV, D], BF16, kind="ExternalInput").ap()
    Vd = nc.dram_tensor("v_tok", [TKV, D], BF16, kind="ExternalInput").ap()
    wq_d = nc.dram_tensor("wq", [D, INNER], BF16, kind="ExternalInput").ap()
    wk_d = nc.dram_tensor("wk", [D, INNER], BF16, kind="ExternalInput").ap()
    wv_d = nc.dram_tensor("wv", [D, INNER], BF16, kind="ExternalInput").ap()
    wo_d = nc.dram_tensor("wo", [INNER, D], BF16, kind="ExternalInput").ap()
    w1_d = nc.dram_tensor("w1", [P, DC * MLP], F8, kind="ExternalInput").ap()
    w2_d = nc.dram_tensor("w2", [P, MC * D], F8, kind="ExternalInput").ap()
    bff1_d = nc.dram_tensor("bff1", [P, MC], F32, kind="ExternalInput").ap()
    brow_d = {}
    for name in ("bq", "bk", "bv", "bo", "b2s"):
        if np.any(bias_rows[name]):
            brow_d[name] = nc.dram_tensor(
                "brow_" + name, [1, bias_rows[name].shape[0]], BF16,
                kind="ExternalInput",
            ).ap()
    Yd = nc.dram_tensor("y", [TQ, D], F32, kind="ExternalOutput").ap()

    Qt = Qd.rearrange("(t p) d -> t p d", p=P)
    Kt = Kd.rearrange("(t p) d -> t p d", p=P)
    Vt = Vd.rearrange("(t p) d -> t p d", p=P)
    Yt = Yd.rearrange("(t p) d -> t p d", p=P)
    wq_v = wq_d.rearrange("(c p) n -> p c n", p=P)
    wk_v = wk_d.rearrange("(c p) n -> p c n", p=P)
    wv_v = wv_d.rearrange("(c p) n -> p c n", p=P)
    wo_v = wo_d.rearrange("(c p) n -> p c n", p=P)
    w1_v = w1_d.rearrange("p (c n) -> p c n", c=DC)
    w2_v = w2_d.rearrange("p (c n) -> p c n", c=MC)

    RG = [[0, 1], [2, 3], [4, 5], [6, 7]]

    with tile.TileContext(nc) as tc:
        with tc.tile_pool(name="singles", bufs=1) as singles:
            wo_sb = singles.tile([P, IC, D], BF16)
            w1_sb = singles.tile([P, DC, MLP], F8)
            w2_sb = singles.tile([P, MC, D], F8)
            bff1_sb = singles.tile([P, MC], F32)
            eps_tile = singles.tile([P, 1], F32)
            nc.vector.memset(eps_tile[:], EPS)
            ones_row = None
            if brow_d:
                ones_row = singles.tile([1, P], BF16)
                nc.vector.memset(ones_row[:], 1.0)
            brow_sb = {}
            for name, ap in brow_d.items():
                t = singles.tile([1, ap.shape[1]], BF16, tag="brow_" + name)
                nc.sync.dma_start(t[:], ap[:])
                brow_sb[name] = t

            x_sb = singles.tile([P, NQT, D], BF16)  # residual / LN2 source
            M_sb = singles.tile([P, IC, P], BF16)  # merged blockdiag M
            nc.vector.memset(M_sb[:], 0.0)
            ffn_rs = singles.tile([P, NQT], F32)
            ffn_nmu = singles.tile([P, NQT], F32)
            Mpack = singles.tile([P, IC, DH], BF16)  # packed local M
            ag_cm = tc.tile_pool(name="ag_dram", bufs=1, space="DRAM")
            ag_pool = ag_cm.__enter__()
            ag_in = ag_pool.tile([P, IC * DH], BF16, name="ag_in")
            ag_out = ag_pool.tile([2 * P, IC * DH], BF16, name="ag_out")

            qnT_cm = tc.tile_pool(name="qnT_pool", bufs=1)
            qnT_pool = qnT_cm.__enter__()
            # feature-major qn for all tiles, pr-major: [P, IC, NQT*P]
            qnT_all = qnT_pool.tile([P, IC, NQT * P], BF16)

            def _ln_stats(pool, x_ap, rs_out=None, nmu_out=None):
                stats = pool.tile([P, 2, 6], F32, tag="ln_stats")
                xv = x_ap.rearrange("p (s f) -> p s f", s=2)
                for s in range(2):
                    nc.vector.bn_stats(out=stats[:, s, :], in_=xv[:, s, :])
                mv = pool.tile([P, 2], F32, tag="ln_mv")
                nc.vector.bn_aggr(out=mv[:], in_=stats[:])
                if rs_out is None:
                    rs_t = pool.tile([P, 1], F32, tag="ln_rs")
                    rs_out = rs_t[:]
                nc.scalar.activation(
                    out=rs_out, in_=mv[:, 1:2], func=AF.Sqrt, bias=eps_tile[:], scale=1.0
                )
                nc.vector.reciprocal(out=rs_out, in_=rs_out)
                if nmu_out is None:
                    nmu_t = pool.tile([P, 1], F32, tag="ln_nmu")
                    nmu_out = nmu_t[:]
                nc.vector.tensor_scalar(
                    out=nmu_out,
                    in0=mv[:, 0:1],
                    scalar1=rs_out,
                    scalar2=-1.0,
                    op0=ALU.mult,
                    op1=ALU.mult,
                )
                return rs_out, nmu_out

            # ===== Phase 1: K/V -> partial M, with Q prep interleaved =====
            with (
                tc.tile_pool(name="kvw", bufs=1) as kvw,
                tc.tile_pool(name="io", bufs=4) as io_pool,
                tc.tile_pool(name="mid", bufs=3) as mid,
                tc.tile_pool(name="nrm", bufs=2) as nrm,
                tc.tile_pool(name="stats", bufs=4) as stats_pool,
                tc.tile_pool(name="pj_ps", bufs=3, space="PSUM") as pj_ps,
                tc.tile_pool(name="m_ps", bufs=1, space="PSUM") as m_ps_pool,
            ):
                # prefetch first k/v tiles ahead of the big weight loads
                pre = {}
                for t in range(2):
                    for which, src in (("k", Kt[t]), ("v", Vt[t])):
                        x_in = io_pool.tile([P, D], BF16, tag="tok_in")
                        nc.gpsimd.dma_start(x_in[:], src[:])
                        pre[(which, t)] = x_in
                wk_sb = kvw.tile([P, DC, INNER], BF16)
                wv_sb = kvw.tile([P, DC, INNER], BF16)
                wq_sb = kvw.tile([P, DC, INNER], BF16)
                nc.gpsimd.dma_start(wk_sb[:, :, 0:512], wk_v[:, :, 0:512])
                nc.gpsimd.dma_start(wk_sb[:, :, 512:1024], wk_v[:, :, 512:1024])
                nc.gpsimd.dma_start(wv_sb[:, :, 0:512], wv_v[:, :, 0:512])
                nc.gpsimd.dma_start(wv_sb[:, :, 512:1024], wv_v[:, :, 512:1024])
                nc.sync.dma_start(bff1_sb[:], bff1_d[:])

                M_ps = m_ps_pool.tile([P, IC, P], F32)
                pending_m = None

                def emit_m(t, kn_bf, v_bf):
                    kn_flat = kn_bf.rearrange("p h f -> p (h f)")
                    for pr in range(IC):
                        nc.tensor.matmul(
                            M_ps[:, pr, :],
                            kn_flat[:, pr * P : (pr + 1) * P],
                            v_bf[:, pr * P : (pr + 1) * P],
                            start=(t == 0 and pr % 4 == 0),
                            stop=(t == NKT - 1 and pr % 4 == 3),
                            skip_group_check=True,
                        )

                def proj(xnT, w_sb, bname):
                    """[tok,1024] projection; returns 2 psum halves."""
                    pss = []
                    for g in range(2):
                        ps = pj_ps.tile([P, 512], F32, tag=f"pj{g}")
                        pss.append(ps)
                    for c in range(DC):
                        for g in range(2):
                            nc.tensor.matmul(
                                pss[g][:],
                                xnT[:, c, :],
                                w_sb[:, c, g * 512 : (g + 1) * 512],
                                start=(c == 0),
                                stop=(c == DC - 1) and (bname not in brow_sb),
                            )
                    for g in range(2):
                        if bname in brow_sb:
                            nc.tensor.matmul(
                                pss[g][:],
                                ones_row[:],
                                brow_sb[bname][:, g * 512 : (g + 1) * 512],
                                start=False,
                                stop=True,
                                skip_group_check=True,
                            )
                    return pss

                def l2norm(pss, out_bf):
                    """per-head l2 normalize psum halves -> out_bf [P,H,DH] bf16"""
                    for g in range(2):
                        sq = mid.tile([P, 512], F32, tag="sq")
                        nc.scalar.activation(out=sq[:], in_=pss[g][:], func=AF.Square)
                        ss = stats_pool.tile([P, 8, 1], F32, tag="l2_ss")
                        nc.vector.reduce_sum(
                            out=ss[:],
                            in_=sq.rearrange("p (h f) -> p h f", h=8),
                            axis=mybir.AxisListType.X,
                        )
                        rn = stats_pool.tile([P, 8, 1], F32, tag="l2_rn")
                        nc.scalar.activation(out=rn[:], in_=ss[:], func=AF.Sqrt)
                        nc.vector.tensor_scalar_max(out=rn[:], in0=rn[:], scalar1=1e-12)
                        nc.vector.reciprocal(out=rn[:], in_=rn[:])
                        nc.vector.tensor_tensor(
                            out=out_bf[:, g * 8 : (g + 1) * 8, :],
                            in0=pss[g].rearrange("p (h f) -> p h f", h=8),
                            in1=rn.to_broadcast([P, 8, DH]),
                            op=ALU.mult,
                        )

                def q_prep(t):
                    """LN + projection + l2 + feature-major transpose for Q tile t.
                    Q was DMA'd into x_sb[:, t, :] (bf16)."""
                    rs, nmu = _ln_stats(stats_pool, x_sb[:, t, :])
                    q_std = mid.tile([P, D], BF16, tag="xn")
                    nc.scalar.activation(
                        out=q_std[:], in_=x_sb[:, t, :], func=AF.Identity,
                        bias=nmu, scale=rs,
                    )
                    qnT = mid.tile([P, DC, P], BF16, tag="xnT")
                    nc.sync.dma_start(qnT[:], q_std[:], transpose=True)
                    pss = proj(qnT, wq_sb, "bq")
                    qn_bf = nrm.tile([P, H, DH], BF16, tag="qn_bf")
                    l2norm(pss, qn_bf)
                    nc.sync.dma_start(
                        qnT_all[:, :, t * P : (t + 1) * P],
                        qn_bf.rearrange("p h f -> p (h f)")[:],
                        transpose=True,
                    )

                for t in range(NKT):
                    # staggered big loads on the gpsimd queue
                    if t == 1:
                        nc.gpsimd.dma_start(wq_sb[:], wq_v[:])
                    elif t == 2:
                        nc.gpsimd.dma_start(wo_sb[:], wo_v[:])
                    elif t == 4:
                        nc.gpsimd.dma_start(w1_sb[:, 0:4, :], w1_v[:, 0:4, :])
                        nc.gpsimd.dma_start(w1_sb[:, 4:8, :], w1_v[:, 4:8, :])
                    elif t == 6:
                        nc.gpsimd.dma_start(w2_sb[:, 0:16, :], w2_v[:, 0:16, :])
                        nc.gpsimd.dma_start(w2_sb[:, 16:32, :], w2_v[:, 16:32, :])
                    kn_bf = None
                    v_bf = None
                    for which in ("k", "v"):
                        if (which, t) in pre:
                            x_in = pre.pop((which, t))
                        else:
                            src = Kt[t] if which == "k" else Vt[t]
                            x_in = io_pool.tile([P, D], BF16, tag="tok_in")
                            nc.gpsimd.dma_start(x_in[:], src[:])
                        w_sb = wk_sb if which == "k" else wv_sb
                        bname = "bk" if which == "k" else "bv"
                        rs, nmu = _ln_stats(stats_pool, x_in[:])
                        xn = mid.tile([P, D], BF16, tag="xn")
                        nc.scalar.activation(
                            out=xn[:], in_=x_in[:], func=AF.Identity, bias=nmu, scale=rs
                        )
                        xnT = mid.tile([P, DC, P], BF16, tag="xnT")
                        nc.sync.dma_start(xnT[:], xn[:], transpose=True)
                        pss = proj(xnT, w_sb, bname)
                        if which == "v":
                            v_bf = nrm.tile([P, INNER], BF16, tag="v_bf")
                            for g in range(2):
                                nc.vector.tensor_copy(
                                    out=v_bf[:, g * 512 : (g + 1) * 512], in_=pss[g][:]
                                )
                        else:
                            kn_bf = nrm.tile([P, H, DH], BF16, tag="kn_bf")
                            l2norm(pss, kn_bf)
                    if pending_m is not None:
                        emit_m(*pending_m)
                    pending_m = (t, kn_bf, v_bf)
                    # queue the Q tile load; Q prep for tiles 0..6 interleaves
                    # here, 7 goes after the AllGather is in flight
                    nc.gpsimd.dma_start(x_sb[:, t, :], Qt[t][:])
                    if t >= 1:
                        q_prep(t - 1)
                emit_m(*pending_m)

                # pack local M (blockdiag halves) and AllGather across the pair
                nc.scalar.activation(
                    out=Mpack[0:64, :, :], in_=M_ps[0:64, :, 0:64], func=AF.Copy
                )
                nc.scalar.activation(
                    out=Mpack[64:128, :, :], in_=M_ps[64:128, :, 64:128], func=AF.Copy
                )
                nc.sync.dma_start(ag_in[:], Mpack.rearrange("p c f -> p (c f)")[:])
                nc.gpsimd.collective_compute(
                    "AllGather",
                    ALU.bypass,
                    replica_groups=RG,
                    ins=[ag_in[:]],
                    outs=[ag_out[:]],
                )
                q_prep(NKT - 1)

            # ===== Phase 2: M merge, attention, x =====
            with (
                tc.tile_pool(name="at_ps", bufs=1, space="PSUM") as at_ps,
                tc.tile_pool(name="x_ps", bufs=1, space="PSUM") as x_ps,
                tc.tile_pool(name="at_mid", bufs=2) as at_mid,
                tc.tile_pool(name="at_stats", bufs=4) as at_stats,
            ):
                both = at_mid.tile([P, 2, IC * DH], BF16, tag="ag_both")
                nc.sync.dma_start(both[:], ag_out.rearrange("(w p) f -> p w f", p=P)[:])
                Msum = at_mid.tile([P, IC, DH], BF16, tag="msum")
                nc.vector.tensor_tensor(
                    out=Msum.rearrange("p c f -> p (c f)")[:],
                    in0=both[:, 0, :],
                    in1=both[:, 1, :],
                    op=ALU.add,
                )
                nc.scalar.activation(
                    out=M_sb[0:64, :, 0:64], in_=Msum[0:64, :, :], func=AF.Copy
                )
                nc.scalar.activation(
                    out=M_sb[64:128, :, 64:128], in_=Msum[64:128, :, :], func=AF.Copy
                )
                ag_cm.__exit__(None, None, None)

                for t2 in range(NQT // 2):
                    a_ps = at_ps.tile([P, IC, 2 * P], F32, tag="attn_ps")
                    for pr in range(IC):
                        nc.tensor.matmul(
                            a_ps[:, pr, :],
                            M_sb[:, pr, :],
                            qnT_all[:, pr, t2 * 2 * P : (t2 + 1) * 2 * P],
                            start=True,
                            stop=True,
                            skip_group_check=True,
                        )
                    aT_bf = at_mid.tile([P, IC, 2 * P], BF16, tag="aT_bf")
                    nc.scalar.activation(out=aT_bf[:], in_=a_ps[:], func=AF.Copy)
                    for tt in range(2):
                        t = t2 * 2 + tt
                        xps = []
                        for g in range(2):
                            ps = x_ps.tile([P, 512], F32, tag=f"x_proj{tt}{g}")
                            xps.append(ps)
                        for c in range(IC):
                            for g in range(2):
                                nc.tensor.matmul(
                                    xps[g][:],
                                    aT_bf[:, c, tt * P : (tt + 1) * P],
                                    wo_sb[:, c, g * 512 : (g + 1) * 512],
                                    start=(c == 0),
                                    stop=(c == IC - 1) and ("bo" not in brow_sb),
                                )
                        for g in range(2):
                            if "bo" in brow_sb:
                                nc.tensor.matmul(
                                    xps[g][:],
                                    ones_row[:],
                                    brow_sb["bo"][:, g * 512 : (g + 1) * 512],
                                    start=False,
                                    stop=True,
                                    skip_group_check=True,
                                )
                            nc.vector.tensor_tensor(
                                out=x_sb[:, t, g * 512 : (g + 1) * 512],
                                in0=xps[g][:],
                                in1=x_sb[:, t, g * 512 : (g + 1) * 512],
                                op=ALU.add,
                            )
                        # FFN LN stats now, while the sqrt table is loaded
                        stats = at_stats.tile([P, 2, 6], F32, tag="ln_stats")
                        xv = x_sb[:, t, :].rearrange("p (s f) -> p s f", s=2)
                        for s in range(2):
                            nc.vector.bn_stats(out=stats[:, s, :], in_=xv[:, s, :])
                        mv = at_stats.tile([P, 2], F32, tag="ln_mv")
                        nc.vector.bn_aggr(out=mv[:], in_=stats[:])
                        nc.scalar.activation(
                            out=ffn_rs[:, t : t + 1], in_=mv[:, 1:2], func=AF.Sqrt,
                            bias=eps_tile[:], scale=1.0,
                        )
                        nc.vector.reciprocal(
                            out=ffn_rs[:, t : t + 1], in_=ffn_rs[:, t : t + 1]
                        )
                        nc.vector.tensor_scalar(
                            out=ffn_nmu[:, t : t + 1],
                            in0=mv[:, 0:1],
                            scalar1=ffn_rs[:, t : t + 1],
                            scalar2=-1.0,
                            op0=ALU.mult,
                            op1=ALU.mult,
                        )
            qnT_cm.__exit__(None, None, None)

            # ===== Phase 3: FFN (fp8 DoubleRow) =====
            with (
                tc.tile_pool(name="f_T", bufs=2) as f_T,
                tc.tile_pool(name="f_mid", bufs=2) as f_mid,
                tc.tile_pool(name="f_h", bufs=2) as f_h,
                tc.tile_pool(name="f_out", bufs=3) as f_out,
                tc.tile_pool(name="h_ps", bufs=2, space="PSUM") as h_ps,
                tc.tile_pool(name="y_ps", bufs=1, space="PSUM") as y_ps,
            ):
                DR = mybir.MatmulPerfMode.DoubleRow
                # hoist LN-apply + transpose + fp8 cast for ALL groups up front
                xnT8s = []
                for grp in range(NG):
                    xnT_bf = f_T.tile([P, DC, GT * P], BF16, tag="xnT_bf")
                    for tt in range(GT):
                        t = grp * GT + tt
                        xn = f_mid.tile([P, D], BF16, tag="f_std")
                        nc.scalar.activation(
                            out=xn[:],
                            in_=x_sb[:, t, :],
                            func=AF.Identity,
                            bias=ffn_nmu[:, t : t + 1],
                            scale=ffn_rs[:, t : t + 1],
                        )
                        nc.sync.dma_start(
                            xnT_bf[:, :, tt * P : (tt + 1) * P], xn[:], transpose=True
                        )
                    xnT8 = f_T.tile([P, DC, GT * P], F8, tag="xnT8")
                    nc.gpsimd.tensor_copy(
                        out=xnT8.rearrange("p c f -> p (c f)")[:],
                        in_=xnT_bf.rearrange("p c f -> p (c f)")[:],
                    )
                    xnT8s.append(xnT8)

                for grp in range(NG):
                    xnT8 = xnT8s[grp]
                    # h^T = gelu((w1q^T @ xnT8)/sw1 + bff1), stored fp8
                    h4 = f_h.tile([P, MC, GT * P], F8, tag="h4")
                    for m in range(MC):
                        hp = h_ps.tile([P, GT * P], F32, tag="h_ps_t")
                        for cp in range(DC // 2):
                            nc.tensor.matmul(
                                hp[:],
                                w1_sb[:, 2 * cp : 2 * cp + 2, m * P : (m + 1) * P],
                                xnT8[:, 2 * cp : 2 * cp + 2, :],
                                start=(cp == 0),
                                stop=(cp == DC // 2 - 1),
                                perf_mode=DR,
                            )
                        nc.scalar.activation(
                            out=h4[:, m, :],
                            in_=hp[:],
                            func=AF.Gelu,
                            bias=bff1_sb[:, m : m + 1],
                            scale=1.0 / sw1,
                        )
                    # y = x + (h @ w2q)/sw2 (+ b2), one 512-col half at a time
                    for g in range(2):
                        yps = [
                            y_ps.tile([P, 512], F32, tag=f"y_ps{tt}",
                                      name=f"yps_{grp}_{g}_{tt}")
                            for tt in range(GT)
                        ]
                        for mp in range(MC // 2):
                            for tt in range(GT):
                                nc.tensor.matmul(
                                    yps[tt][:],
                                    h4[:, 2 * mp : 2 * mp + 2, tt * P : (tt + 1) * P],
                                    w2_sb[:, 2 * mp : 2 * mp + 2, g * 512 : (g + 1) * 512],
                                    start=(mp == 0),
                                    stop=(mp == MC // 2 - 1) and ("b2s" not in brow_sb),
                                    perf_mode=DR,
                                )
                        for tt in range(GT):
                            t = grp * GT + tt
                            if "b2s" in brow_sb:
                                nc.tensor.matmul(
                                    yps[tt][:],
                                    ones_row[:],
                                    brow_sb["b2s"][:, g * 512 : (g + 1) * 512],
                                    start=False,
                                    stop=True,
                                    skip_group_check=True,
                                )
                            y_out = f_out.tile([P, 512], F32, tag="y_out")
                            nc.vector.scalar_tensor_tensor(
                                out=y_out[:],
                                in0=yps[tt][:],
                                scalar=1.0 / sw2,
                                in1=x_sb[:, t, g * 512 : (g + 1) * 512],
                                op0=ALU.mult,
                                op1=ALU.add,
                            )
                            nc.sync.dma_start(
                                Yt[t][:, g * 512 : (g + 1) * 512], y_out[:]
                            )

    nc.compile()
    return nc


def prep_inputs(inputs):
    """Host-side shard + weight folding/quantization.
    Returns (in_maps, bias_rows, sw1, sw2)."""
    f32 = np.float32
    bf = ml_dtypes.bfloat16
    e4 = ml_dtypes.float8_e4m3
    g1 = np.asarray(inputs["ln1_g"], f32)
    b1ln = np.asarray(inputs["ln1_b"], f32)
    g2 = np.asarray(inputs["ln2_g"], f32)
    b2ln = np.asarray(inputs["ln2_b"], f32)
    wq = np.asarray(inputs["wq"], f32)
    wk = np.asarray(inputs["wk"], f32)
    wv = np.asarray(inputs["wv"], f32)
    wo = np.asarray(inputs["wo"], f32)
    w1 = np.asarray(inputs["w1"], f32)
    w2 = np.asarray(inputs["w2"], f32)

    w1g = g2[:, None] * w1
    sw1 = float(F8MAX / max(np.abs(w1g).max(), 1e-30))
    sw2 = float(F8MAX / max(np.abs(w2).max(), 1e-30))

    bias_rows = {
        "bq": (b1ln @ wq).astype(f32),
        "bk": (b1ln @ wk).astype(f32),
        "bv": (b1ln @ wv).astype(f32),
        "bo": np.asarray(inputs["bo"], f32),
        "b2s": (np.asarray(inputs["b2"], f32) * sw2).astype(f32),
    }
    bff1 = (b2ln @ w1 + np.asarray(inputs["b1"], f32)).astype(f32)
    bff1_tile = np.ascontiguousarray(bff1.reshape(MC, P).T)  # [P, MC]

    wq_b = np.ascontiguousarray((g1[:, None] * wq).astype(bf))
    wk_b = np.ascontiguousarray((g1[:, None] * wk).astype(bf))
    wv_b = np.ascontiguousarray((g1[:, None] * wv).astype(bf))
    wo_b = np.ascontiguousarray(wo.astype(bf))
    # fp8 packed weights:
    #   w1p[p, c*MLP + j] = w1g[c*128 + p, j] * sw1
    #   w2p[p, m*D + j]   = w2[m*128 + p, j] * sw2
    w1q = np.clip(w1g * sw1, -240.0, 240.0).astype(e4)
    w1p = np.ascontiguousarray(
        w1q.reshape(DC, P, MLP).transpose(1, 0, 2).reshape(P, DC * MLP)
    )
    w2q = np.clip(w2 * sw2, -240.0, 240.0).astype(e4)
    w2p = np.ascontiguousarray(
        w2q.reshape(MC, P, D).transpose(1, 0, 2).reshape(P, MC * D)
    )

    Q = np.asarray(inputs["Q"], f32)
    K = np.asarray(inputs["K"], f32)
    V = np.asarray(inputs["V"], f32)

    in_maps = []
    for c in range(NCORES):
        b = c // 2
        r0 = (c % 2) * TQ
        m = {
            "q_tok": np.ascontiguousarray(Q[b, r0 : r0 + TQ].astype(bf)),
            "k_tok": np.ascontiguousarray(K[b, r0 : r0 + TKV].astype(bf)),
            "v_tok": np.ascontiguousarray(V[b, r0 : r0 + TKV].astype(bf)),
            "wq": wq_b,
            "wk": wk_b,
            "wv": wv_b,
            "wo": wo_b,
            "w1": w1p,
            "w2": w2p,
            "bff1": bff1_tile,
        }
        for name, row in bias_rows.items():
            if np.any(row):
                m["brow_" + name] = row[None, :].astype(bf)
        in_maps.append(m)
    return in_maps, bias_rows, sw1, sw2


_NC_CACHE = {}


def kernel(**inputs) -> np.ndarray:
    from concourse.bass_utils import run_bass_kernel_spmd

    in_maps, bias_rows, sw1, sw2 = prep_inputs(inputs)
    key = (
        tuple(sorted(n for n, r in bias_rows.items() if np.any(r))),
        round(sw1, 9),
        round(sw2, 9),
    )
    if key not in _NC_CACHE:
        _NC_CACHE[key] = build_nc(bias_rows, sw1, sw2)
    nc = _NC_CACHE[key]
    res = run_bass_kernel_spmd(nc, in_maps, core_ids=list(range(NCORES)))
    out = np.empty((B, N, D), np.float32)
    for c in range(NCORES):
        b = c // 2
        r0 = (c % 2) * TQ
        out[b, r0 : r0 + TQ] = res.results[c]["y"]
    return out
